# revision 2
# baseline (speedup 1.0000x reference)
"""Distributed causal-attention kernel for one TRN2 chip (8 NeuronCores).

Problem: x[4, 4096, 1024], single-head causal attention with d_model=1024.
  q/k/v = x @ W^T ; scores = q k^T / sqrt(d) ; causal mask ; softmax ; out = p v

Sharding: 8 cores = 4 batches x 2 q-groups. Every core computes K/V for its
whole batch (4096 tokens) and attends a balanced, SPMD-identical schedule of
16 q-tiles (128 rows each). Global q-tile j (span 128(j+1)) is assigned so
core-half h takes j = 2l + h for local slot l; every slot computes the same
scheduled span 256(l+1) and a per-core [128, 256] bias tile (data, not code)
applies the causal boundary, keeping one instruction stream across all cores.

Matmuls run in bf16 (f32 PSUM accumulation); softmax in f32 on-chip.
"""

import sys

sys.path.insert(0, "/opt/trn_rl_repo")

import numpy as np
import ml_dtypes

B, S, D = 4, 4096, 1024
P = 128              # partition dim
DC = D // P          # 8 contraction chunks
NSLOT = 16           # q-tiles per core
QLOC = NSLOT * P     # 2048 q rows per core
NEG = -1.0e30
SCALE = 1.0 / 32.0   # 1/sqrt(1024)
BF16 = ml_dtypes.bfloat16

_NC = None           # cached compiled graph


def _build():
    import concourse.tile as tile
    from concourse import bacc, mybir
    from concourse.masks import make_identity

    f32, bf16 = mybir.dt.float32, mybir.dt.bfloat16
    X = mybir.AxisListType.X
    Exp = mybir.ActivationFunctionType.Exp
    Copy = mybir.ActivationFunctionType.Copy

    nc = bacc.Bacc("TRN2", target_bir_lowering=False, debug=False)
    xt_d = nc.dram_tensor("xt", [D, S], bf16, kind="ExternalInput")
    xqt_d = nc.dram_tensor("xqt", [D, QLOC], bf16, kind="ExternalInput")
    wq_d = nc.dram_tensor("wq", [D, D], bf16, kind="ExternalInput")
    wk_d = nc.dram_tensor("wk", [D, D], bf16, kind="ExternalInput")
    wv_d = nc.dram_tensor("wv", [D, D], bf16, kind="ExternalInput")
    cb_d = nc.dram_tensor("cbias", [P, 256], f32, kind="ExternalInput")
    out_d = nc.dram_tensor("out", [QLOC, D], f32, kind="ExternalOutput")

    xt_r = xt_d[:].rearrange("(c p) n -> p c n", p=P)
    xqt_r = xqt_d[:].rearrange("(c p) n -> p c n", p=P)
    wq_r = wq_d[:].rearrange("(c p) n -> p c n", p=P)
    wk_r = wk_d[:].rearrange("(c p) n -> p c n", p=P)
    wv_r = wv_d[:].rearrange("(c p) n -> p c n", p=P)

    with tile.TileContext(nc) as tc:
        with tc.tile_pool(name="resid", bufs=1) as resid, \
             tc.tile_pool(name="consts", bufs=1) as consts, \
             tc.tile_pool(name="stats", bufs=4) as stats:
            KT = resid.tile([P, DC, S], bf16)          # K^T  [d, keys]
            V = resid.tile([P, S // P, D], bf16)       # V    [keys, d]
            QT = resid.tile([P, DC, QLOC], bf16)       # Q^T  [d, q]
            ident = consts.tile([P, P], bf16)
            make_identity(nc, ident[:])
            cb = consts.tile([P, 256], f32)
            nc.sync.dma_start(cb[:], cb_d[:])

            # ---------------- phase 1: projections ----------------
            with tc.tile_pool(name="xs", bufs=2) as xs, \
                 tc.tile_pool(name="wp", bufs=1) as wp, \
                 tc.tile_pool(name="pp1", bufs=4, space="PSUM") as pp1:
                # K^T sweep
                wk = wp.tile([P, DC, D], bf16, tag="w", name="wk_sb")
                nc.sync.dma_start(wk[:], wk_r)
                for tb in range(S // 512):
                    xb = xs.tile([P, DC, 512], bf16, tag="x", name="xb_k")
                    nc.sync.dma_start(xb[:], xt_r[:, :, tb * 512:(tb + 1) * 512])
                    for do in range(DC):
                        ps = pp1.tile([P, 512], f32, tag="ps1", name="ps_k")
                        for c in range(DC):
                            nc.tensor.matmul(
                                ps[:], wk[:, c, do * P:(do + 1) * P], xb[:, c, :],
                                start=(c == 0), stop=(c == DC - 1))
                        nc.vector.tensor_copy(
                            KT[:, do, tb * 512:(tb + 1) * 512], ps[:])
                # V sweep
                wv = wp.tile([P, DC, D], bf16, tag="w", name="wv_sb")
                nc.sync.dma_start(wv[:], wv_r)
                for tb in range(S // 512):
                    xb = xs.tile([P, DC, 512], bf16, tag="x", name="xb_v")
                    nc.sync.dma_start(xb[:], xt_r[:, :, tb * 512:(tb + 1) * 512])
                    for tq in range(4):
                        tch = tb * 4 + tq
                        for dv in range(2):
                            ps = pp1.tile([P, 512], f32, tag="ps1", name="ps_v")
                            for c in range(DC):
                                nc.tensor.matmul(
                                    ps[:], xb[:, c, tq * P:(tq + 1) * P],
                                    wv[:, c, dv * 512:(dv + 1) * 512],
                                    start=(c == 0), stop=(c == DC - 1))
                            nc.vector.tensor_copy(
                                V[:, tch, dv * 512:(dv + 1) * 512], ps[:])
                # Q^T sweep
                wq = wp.tile([P, DC, D], bf16, tag="w", name="wq_sb")
                nc.sync.dma_start(wq[:], wq_r)
                for tb in range(QLOC // 512):
                    xb = xs.tile([P, DC, 512], bf16, tag="x", name="xb_q")
                    nc.sync.dma_start(xb[:], xqt_r[:, :, tb * 512:(tb + 1) * 512])
                    for do in range(DC):
                        ps = pp1.tile([P, 512], f32, tag="ps1", name="ps_q")
                        for c in range(DC):
                            nc.tensor.matmul(
                                ps[:], wq[:, c, do * P:(do + 1) * P], xb[:, c, :],
                                start=(c == 0), stop=(c == DC - 1))
                        nc.scalar.copy(QT[:, do, tb * 512:(tb + 1) * 512], ps[:])

            # ---------------- phase 2: attention ----------------
            with tc.tile_pool(name="scp", bufs=2) as scp, \
                 tc.tile_pool(name="ptp", bufs=2) as ptp, \
                 tc.tile_pool(name="osb", bufs=2) as osb, \
                 tc.tile_pool(name="psc", bufs=2, space="PSUM") as psc, \
                 tc.tile_pool(name="pst", bufs=2, space="PSUM") as pst, \
                 tc.tile_pool(name="pso", bufs=4, space="PSUM") as pso:
                for l in range(NSLOT):
                    span = 256 * (l + 1)
                    nkc = span // P
                    chunks = []
                    off = 0
                    while off < span:
                        w = min(512, span - off)
                        chunks.append((off, w))
                        off += w
                    sc = scp.tile([P, S], bf16, tag="scores", name="sc")
                    chm = stats.tile([P, 8], f32, tag="chm", name="chm")
                    for j, (off, w) in enumerate(chunks):
                        ps = psc.tile([P, 512], f32, tag="psc", name="ps_s")
                        for c in range(DC):
                            nc.tensor.matmul(
                                ps[:, :w], QT[:, c, l * P:(l + 1) * P],
                                KT[:, c, off:off + w],
                                start=(c == 0), stop=(c == DC - 1))
                        if j == len(chunks) - 1:
                            nc.vector.tensor_add(
                                ps[:, w - 256:w], ps[:, w - 256:w], cb[:])
                        nc.vector.reduce_max(chm[:, j:j + 1], ps[:, :w], axis=X)
                        nc.vector.tensor_copy(sc[:, off:off + w], ps[:, :w])
                    rmax = stats.tile([P, 1], f32, tag="rmax", name="rmax")
                    nc.vector.reduce_max(rmax[:], chm[:, :len(chunks)], axis=X)
                    negb = stats.tile([P, 1], f32, tag="negb", name="negb")
                    nc.vector.tensor_scalar_mul(negb[:], rmax[:], -SCALE)
                    rsum = stats.tile([P, 1], f32, tag="rsum", name="rsum")
                    nc.scalar.activation(
                        sc[:, :span], sc[:, :span], Exp,
                        bias=negb[:], scale=SCALE, accum_out=rsum[:])
                    pt = ptp.tile([P, S // P, P], bf16, tag="pt", name="pt")
                    for kc in range(nkc):
                        tp = pst.tile([P, P], bf16, tag="pst", name="tp")
                        nc.tensor.transpose(
                            tp[:], sc[:, kc * P:(kc + 1) * P], ident[:])
                        if kc % 2 == 0:
                            nc.vector.tensor_copy(pt[:, kc, :], tp[:])
                        else:
                            nc.scalar.copy(pt[:, kc, :], tp[:])
                    o0 = pso.tile([P, 512], f32, tag="pso", name="o0")
                    o1 = pso.tile([P, 512], f32, tag="pso", name="o1")
                    opair = (o0, o1)
                    for kc in range(nkc):
                        for dv in range(2):
                            nc.tensor.matmul(
                                opair[dv][:], pt[:, kc, :],
                                V[:, kc, dv * 512:(dv + 1) * 512],
                                start=(kc == 0), stop=(kc == nkc - 1))
                    rec = stats.tile([P, 1], f32, tag="rec", name="rec")
                    nc.vector.reciprocal(rec[:], rsum[:])
                    ob = osb.tile([P, D], f32, tag="ob", name="ob")
                    for dv in range(2):
                        nc.scalar.activation(
                            ob[:, dv * 512:(dv + 1) * 512], opair[dv][:], Copy,
                            scale=rec[:])
                    nc.sync.dma_start(out_d[l * P:(l + 1) * P, :], ob[:])
    nc.compile()
    return nc


def _get_nc():
    global _NC
    if _NC is None:
        _NC = _build()
    return _NC


def _qrows(h):
    """Global q-row indices handled by core-half h, in local order."""
    idx = []
    for l in range(NSLOT):
        j = 2 * l + h
        idx.append(np.arange(j * P, (j + 1) * P))
    return np.concatenate(idx)


def _cbias(h):
    tri = np.where(np.arange(P)[None, :] <= np.arange(P)[:, None],
                   np.float32(0.0), np.float32(NEG)).astype(np.float32)
    if h == 0:
        return np.concatenate([tri, np.full((P, P), NEG, np.float32)], axis=1)
    return np.concatenate([np.zeros((P, P), np.float32), tri], axis=1)


def _is_tril(mask):
    m = np.asarray(mask)
    if m.shape != (S, S):
        return False
    return bool(np.array_equal(m != 0, np.tril(np.ones((S, S), bool))))


def _reference_np(x, w_q, w_k, w_v, mask):
    out = np.empty((B, S, D), np.float32)
    maskz = (np.asarray(mask) == 0)
    for b in range(B):
        q = x[b] @ w_q.T
        k = x[b] @ w_k.T
        v = x[b] @ w_v.T
        s = (q @ k.T) * np.float32(SCALE)
        s[maskz] = -np.inf
        s -= s.max(axis=-1, keepdims=True)
        np.exp(s, out=s)
        s /= s.sum(axis=-1, keepdims=True)
        out[b] = s @ v
    return out


def kernel(x, w_q, w_k, w_v, mask):
    x = np.asarray(x, np.float32)
    w_q = np.asarray(w_q, np.float32)
    w_k = np.asarray(w_k, np.float32)
    w_v = np.asarray(w_v, np.float32)

    if not _is_tril(mask):
        # Mask is not the expected causal tril: fall back to a host reference.
        return _reference_np(x, w_q, w_k, w_v, mask)

    from concourse.bass_utils import run_bass_kernel_spmd

    nc = _get_nc()
    wq_t = np.ascontiguousarray(w_q.T).astype(BF16)
    wk_t = np.ascontiguousarray(w_k.T).astype(BF16)
    wv_t = np.ascontiguousarray(w_v.T).astype(BF16)
    in_maps = []
    rows = [_qrows(0), _qrows(1)]
    cbs = [_cbias(0), _cbias(1)]
    for c in range(8):
        b, h = c // 2, c % 2
        xt = np.ascontiguousarray(x[b].T).astype(BF16)
        xqt = np.ascontiguousarray(x[b][rows[h]].T).astype(BF16)
        in_maps.append({
            "xt": xt, "xqt": xqt,
            "wq": wq_t, "wk": wk_t, "wv": wv_t,
            "cbias": cbs[h],
        })
    res = run_bass_kernel_spmd(nc, in_maps, list(range(8)))
    out = np.empty((B, S, D), np.float32)
    for c in range(8):
        b, h = c // 2, c % 2
        out[b, rows[h]] = res.results[c]["out"]
    return out
